# revision 14
# baseline (speedup 1.0000x reference)
"""CRF loss kernel for Trainium2 (8 NeuronCores, data-parallel over batch).

Math (faithful to the reference):
  loss = (forscore - tg_energy) / B
  tg_energy = B*trans[0,START] + sum_bt scores[b,t,0] + sum_bt trans[0, gold[b,t]]
    (the reference's torch.gather-on-flattened-(L*L) quirk reduces to row 0)
  forscore = sum_b fs_T[b, END], where fs is the standard CRF forward recurrence
    fs_{t+1}[j] = logsumexp_i(fs_t[i] + scores[t,i] + trans[i,j]), fs_0 = trans[START,:]

Device algorithm: linear-space recurrence w <- E^T (w * exp(scores_t - delta))
with E = exp(trans) in bf16 (PSUM accumulation stays f32). Per-batch magnitude
drift is measured every K steps via a ones-vector matmul (z = 1^T y); 1/z is fed
back (with a 2-chunk pipeline delay) as a one-step multiplicative correction on
the first exp'd-score tile of a later chunk, keeping |log w| bounded. The z
values stream to DRAM; the host adds back sum(log z) over applied corrections:
  fs_T[b] = log w_T[END,b] + sum_c log z[c,b] + T*delta.

Per-core layout: tags on partitions (48), local batch (8) on the free dim.
The per-step matmul keeps E stationary: w'(48j,8b) = matmul(lhsT=E(48i,48j), rhs=y(48i,8b)).

The scalar engine runs ONLY Exp activations (no function-table thrashing);
feedback uses the vector-engine reciprocal instead of Ln/Exp.

mask is all ones per the problem spec (fill: ones), so the mask gating
(where(mask, nxt, fs)) is the identity and is not materialized on device.
"""

import numpy as np

B, T, L = 64, 512, 48
START, PAD, END = 46, 45, 47
NCORES = 8
BL = B // NCORES          # 8 batch elements per core
K = 16                    # steps per renorm chunk
NCH = T // K              # 32 chunks
LAG = 2                   # feedback delay (chunks) for the 1/z correction
DELTA = 5.0               # static per-step log shift folded into exp(scores)

_NC_CACHE = {}


def build_nc():
    import concourse.bacc as bacc
    import concourse.mybir as mybir
    import concourse.tile as tile

    f32 = mybir.dt.float32
    bf16 = mybir.dt.bfloat16
    AF = mybir.ActivationFunctionType
    AL = mybir.AluOpType
    AX = mybir.AxisListType

    nc = bacc.Bacc("TRN2", target_bir_lowering=False, debug=False)

    s_dram = nc.dram_tensor("s_tr", [L, T * BL], f32, kind="ExternalInput")
    trans_d = nc.dram_tensor("trans", [L, L], f32, kind="ExternalInput")
    transT_d = nc.dram_tensor("transT", [L, L], f32, kind="ExternalInput")
    goldf_d = nc.dram_tensor("goldf", [128, 32], f32, kind="ExternalInput")
    sc0_d = nc.dram_tensor("sc0", [128, 32], f32, kind="ExternalInput")
    iota_d = nc.dram_tensor("iotaf", [128, L], f32, kind="ExternalInput")

    w_out_d = nc.dram_tensor("w_out", [L, BL], f32, kind="ExternalOutput")
    zlog_d = nc.dram_tensor("z_out", [1, NCH * BL], f32, kind="ExternalOutput")
    scal_d = nc.dram_tensor("scalars_out", [1, 2], f32, kind="ExternalOutput")

    with tile.TileContext(nc) as tc:
        with (
            tc.tile_pool(name="const", bufs=1) as cpool,
            tc.tile_pool(name="sraw", bufs=3) as rpool,
            tc.tile_pool(name="sexp", bufs=3) as epool,
            tc.tile_pool(name="yy", bufs=3) as ypool,
            tc.tile_pool(name="small", bufs=4) as smpool,
            tc.tile_pool(name="oh", bufs=2) as ohpool,
            tc.tile_pool(name="wps", bufs=2, space="PSUM") as wpool,
            tc.tile_pool(name="cbps", bufs=2, space="PSUM") as cbpool,
            tc.tile_pool(name="zps", bufs=2, space="PSUM") as zpool,
            tc.tile_pool(name="cntps", bufs=1, space="PSUM") as cntpool,
            tc.tile_pool(name="tgps", bufs=1, space="PSUM") as tgpool,
        ):
            # ---- constants ----
            trans_sb = cpool.tile([L, L], f32)
            nc.sync.dma_start(trans_sb[:], trans_d[:])
            transT_sb = cpool.tile([L, L], f32)
            nc.sync.dma_start(transT_sb[:], transT_d[:])
            iota_sb = cpool.tile([128, L], f32)
            nc.sync.dma_start(iota_sb[:], iota_d[:])
            goldf_sb = cpool.tile([128, 32], f32)
            nc.sync.dma_start(goldf_sb[:], goldf_d[:])
            sc0_sb = cpool.tile([128, 32], f32)
            nc.sync.dma_start(sc0_sb[:], sc0_d[:])

            zero48 = cpool.tile([L, 1], f32)
            nc.vector.memset(zero48[:], 0.0)
            negd48 = cpool.tile([L, 1], f32)
            nc.vector.memset(negd48[:], -DELTA)

            E_bf = cpool.tile([L, L], bf16)     # exp(trans), bf16 stationary
            nc.scalar.activation(E_bf[:], trans_sb[:], AF.Exp, bias=zero48[:])
            ET2_sb = cpool.tile([L, L], f32)    # exp(trans[j, i]) at [i, j]
            nc.scalar.activation(ET2_sb[:], transT_sb[:], AF.Exp, bias=zero48[:])

            ones48b = cpool.tile([L, 1], bf16)
            nc.vector.memset(ones48b[:], 1.0)
            ones1x48 = cpool.tile([1, L], bf16)
            nc.vector.memset(ones1x48[:], 1.0)
            ones128b = cpool.tile([128, 1], bf16)
            nc.vector.memset(ones128b[:], 1.0)
            ones128f = cpool.tile([128, 1], f32)
            nc.vector.memset(ones128f[:], 1.0)
            zbuf = cpool.tile([1, NCH * BL], f32)

            # PE clock warm-up: ~3.5us of back-to-back matmuls releases the
            # HAM throttle (1.2 -> 2.4 GHz); the chain's ~250ns PE gaps never
            # re-throttle it afterwards.
            for _ in range(64):
                warm_ps = wpool.tile([L, L], f32, tag="w")
                nc.tensor.matmul(
                    warm_ps[:], E_bf[:], E_bf[:], start=True, stop=True)

            # ---- main recurrence ----
            cb_tiles = {}
            w_prev = None
            y = None
            for c in range(NCH):
                raw = rpool.tile([L, K * BL], f32)
                nc.sync.dma_start(raw[:], s_dram[:, c * K * BL:(c + 1) * K * BL])
                se = epool.tile([L, K, BL], f32, tag="se")
                nc.scalar.activation(
                    se[:].rearrange("p a b -> p (a b)"), raw[:], AF.Exp,
                    bias=negd48[:])
                if c in cb_tiles:
                    cbt = cb_tiles.pop(c)
                    s0c = smpool.tile([L, BL], f32, tag="s0c")
                    nc.vector.tensor_tensor(s0c[:], se[:, 0, :], cbt[:], AL.mult)
                else:
                    s0c = None

                for k in range(K):
                    t = c * K + k
                    s_sl = s0c[:] if (k == 0 and s0c is not None) else se[:, k, :]
                    y = ypool.tile([L, BL], bf16, tag="y")
                    if t == 0:
                        nc.vector.tensor_scalar_mul(
                            y[:], s_sl, ET2_sb[:, START:START + 1])
                    else:
                        nc.vector.tensor_tensor(y[:], w_prev[:], s_sl, AL.mult)
                    w_ps = wpool.tile([L, BL], f32, tag="w")
                    nc.tensor.matmul(w_ps[:], E_bf[:], y[:], start=True, stop=True)
                    w_prev = w_ps

                # chunk-end magnitude measurement + delayed 1/z feedback
                z_ps = zpool.tile([1, BL], f32, tag="z")
                nc.tensor.matmul(z_ps[:], ones48b[:], y[:], start=True, stop=True)
                nc.vector.tensor_copy(zbuf[:, c * BL:(c + 1) * BL], z_ps[:])
                if c + LAG < NCH:
                    zr = smpool.tile([1, BL], bf16, tag="zr")
                    # bf16 rounding of the 1/z feedback factor is accounted
                    # exactly on the host (it only shifts which factor was
                    # applied; log z is added back from the f32 z_out).
                    with nc.allow_low_precision(reason="renorm feedback factor"):
                        nc.vector.reciprocal(zr[:], z_ps[:])
                    cbt = cbpool.tile([L, BL], f32, tag="cb")
                    nc.tensor.matmul(
                        cbt[:], ones1x48[:], zr[:], start=True, stop=True)
                    cb_tiles[c + LAG] = cbt

            w_sb = smpool.tile([L, BL], f32, tag="wout")
            nc.vector.tensor_copy(w_sb[:], w_prev[:])
            nc.sync.dma_start(w_out_d[:], w_sb[:])
            nc.sync.dma_start(zlog_d[:], zbuf[:])

            # ---- gold-tag histogram:  cnt[v] = sum_n [gold[n] == v] ----
            cnt_ps = cntpool.tile([L, 1], f32)
            for cc in range(32):
                oh = ohpool.tile([128, L], bf16, tag="oh")
                nc.vector.tensor_scalar(
                    oh[:], iota_sb[:], goldf_sb[:, cc:cc + 1], None, AL.is_equal)
                nc.tensor.matmul(
                    cnt_ps[:], oh[:], ones128b[:],
                    start=(cc == 0), stop=(cc == 31))
            cnt_sb = smpool.tile([L, 1], f32, tag="cnt")
            nc.vector.tensor_copy(cnt_sb[:], cnt_ps[:])
            tg_ps = tgpool.tile([1, 2], f32)
            # tg_gather = sum_v cnt[v] * trans[0, v]   (trans0 column = transT[:, 0])
            nc.tensor.matmul(
                tg_ps[:, 0:1], cnt_sb[:], transT_sb[:, 0:1], start=True, stop=True)
            # sc0_sum = sum scores[:, :, 0]
            red = smpool.tile([128, 1], f32, tag="red")
            nc.vector.reduce_sum(red[:], sc0_sb[:], axis=AX.X)
            nc.tensor.matmul(
                tg_ps[:, 1:2], red[:], ones128f[:], start=True, stop=True)
            scal_sb = smpool.tile([1, 2], f32, tag="scal")
            nc.vector.tensor_copy(scal_sb[:], tg_ps[:])
            nc.sync.dma_start(scal_d[:], scal_sb[:])

    nc.compile()
    return nc


def _get_nc():
    if "nc" not in _NC_CACHE:
        _NC_CACHE["nc"] = build_nc()
    return _NC_CACHE["nc"]


def make_in_maps(scores, gold_target, transitions):
    scores = np.asarray(scores, dtype=np.float32)
    gold = np.asarray(gold_target)
    trans = np.ascontiguousarray(np.asarray(transitions, dtype=np.float32))
    transT = np.ascontiguousarray(trans.T)
    iota = np.ascontiguousarray(
        np.broadcast_to(np.arange(L, dtype=np.float32)[None, :], (128, L)))
    in_maps = []
    for c in range(NCORES):
        sc = scores[c * BL:(c + 1) * BL]                     # (BL, T, L)
        s_tr = np.ascontiguousarray(sc.transpose(2, 1, 0)).reshape(L, T * BL)
        goldf = np.ascontiguousarray(
            gold[c * BL:(c + 1) * BL].astype(np.float32).reshape(128, 32))
        sc0 = np.ascontiguousarray(sc[:, :, 0].astype(np.float32).reshape(128, 32))
        in_maps.append({
            "s_tr": s_tr, "trans": trans, "transT": transT,
            "goldf": goldf, "sc0": sc0, "iotaf": iota,
        })
    return in_maps


def combine_outputs(results, transitions):
    trans = np.asarray(transitions, dtype=np.float64)
    forscore = 0.0
    tg_energy = 0.0
    for c in range(NCORES):
        r = results[c]
        w = np.asarray(r["w_out"], dtype=np.float64)            # (L, BL)
        zv = np.asarray(r["z_out"], dtype=np.float64).reshape(NCH, BL)
        scal = np.asarray(r["scalars_out"], dtype=np.float64)[0]
        fs_end = (np.log(w[END, :]) + np.log(zv[:NCH - LAG]).sum(axis=0)
                  + DELTA * T)
        forscore += fs_end.sum()
        tg_energy += scal[0] + scal[1] + BL * trans[0, START]
    return np.float32((forscore - tg_energy) / B)


def kernel(scores, gold_target, mask, transitions):
    from concourse.bass_utils import run_bass_kernel_spmd

    nc = _get_nc()
    in_maps = make_in_maps(scores, gold_target, transitions)
    res = run_bass_kernel_spmd(nc, in_maps, list(range(NCORES)))
    return combine_outputs(res.results, transitions)
